# revision 24
# baseline (speedup 1.0000x reference)
"""Trainium2 Bass kernel for the Memoroid linear-recurrence block.

Math (per batch b, fp32):
    a = sigmoid(x @ W_a + b_a)          [T, D]
    bm = x @ W_b                        [T, D]
    h_t = a_t * h_{t-1} + bm_t          (h_{-1} = h0, scan over t)
    y = gelu_tanh(h) @ W_y + x @ W_skip [T, D]
Returns (h, y).

Strategy: data-parallel over batch (8 sequences -> 8 cores). All on-chip
work happens in the transposed [d, t] orientation so the recurrence maps
onto the DVE tensor_tensor_scan instruction and NO PE transposes are
needed anywhere:
  - the host ships x already transposed (and cast to bf16) as
    [128 part, 8 k, T] so it is directly usable as the matmul moving
    operand ([d_in, t] tiles),
  - a/b are computed as [d_h, t] tiles (weights stationary),
  - y is computed transposed as well: yT[d_out, t] = W_y^T gelu(h)T +
    W_skip^T xT, consuming the scan output gT in its native layout,
  - h/y are stored transposed+bf16 to DRAM and the host transposes the
    fp32 result back.
All matmuls run in bf16 (1 cyc/row, fast weight loads); PSUM accumulates
fp32 and the scan carry stays fp32, so the end-to-end max rel-err is
~3e-3 (dominated by bf16 input rounding).
"""

import sys

for _p in ("/opt/trn_rl_repo",):
    if _p not in sys.path:
        sys.path.insert(0, _p)

from contextlib import ExitStack

import numpy as np
import ml_dtypes

import concourse.bass as bass
import concourse.bacc as bacc
import concourse.mybir as mybir
from concourse import tile
from concourse.bass_utils import run_bass_kernel_spmd

B, T, D = 8, 4096, 1024
P = 128
KT = D // P            # 8 partition tiles along any d-dimension
TB = 512               # time-block length (matmul free dim / scan length)
NB = T // TB           # 8 blocks

f32 = mybir.dt.float32
bf16 = mybir.dt.bfloat16

_CACHE = {}


def _build():
    nc = bacc.Bacc()

    # x pre-transposed on host, block-contiguous so each block is one DMA
    # of 128 descriptors x 8KB: xt[p, n, k, t] = x[n*TB+t, k*128+p], bf16
    xt_d = nc.declare_dram_parameter("xt", [P, NB, KT, TB], bf16, False)
    # a/b weights pre-tiled on host OUTPUT-block-major so the j=0 block is
    # one small DMA and the PE can start after ~1.25 MB instead of 5 MB:
    #   w[p, j, k*128+q] = W[k*128+p, j*128+q]
    wa_d = nc.declare_dram_parameter("wa", [P, KT, D], bf16, False)
    wb_d = nc.declare_dram_parameter("wb", [P, KT, D], bf16, False)
    # y/skip weights contraction-major (loaded later, one DMA each):
    #   w[p, k, e] = W[k*128+p, e]
    wy_d = nc.declare_dram_parameter("wy", [P, KT, D], bf16, False)
    ws_d = nc.declare_dram_parameter("ws", [P, KT, D], bf16, False)
    # bias/initial state tiled [p, j] fp32
    ba_d = nc.declare_dram_parameter("ba", [P, KT], f32, False)
    h0_d = nc.declare_dram_parameter("h0", [P, KT], f32, False)
    # outputs transposed: [p, j, t] bf16; host transposes back
    h_d = nc.declare_dram_parameter("h_out", [P, KT, T], bf16, True)
    y_d = nc.declare_dram_parameter("y_out", [P, KT, T], bf16, True)

    AF = mybir.ActivationFunctionType
    ALU = mybir.AluOpType

    with tile.TileContext(nc) as tc, ExitStack() as ctx:
        wpool = ctx.enter_context(tc.tile_pool(name="weights", bufs=1))
        const_pool = ctx.enter_context(tc.tile_pool(name="const", bufs=1))
        xt_pool = ctx.enter_context(tc.tile_pool(name="xt", bufs=3))
        sc_pool = ctx.enter_context(tc.tile_pool(name="scan", bufs=2))
        st_pool = ctx.enter_context(tc.tile_pool(name="stage", bufs=2))
        ps_ab = ctx.enter_context(tc.tile_pool(name="ab", bufs=2, space="PSUM"))
        ps_y = ctx.enter_context(tc.tile_pool(name="ypsum", bufs=3, space="PSUM"))

        # --- PE warm-up: short matmuls on zeroed scratch, no DMA deps, so
        # the HAM clock gate is at 2.4 GHz by the time real matmuls start.
        # FD=128 keeps each one ~107ns so the queue drains before real
        # work is ready. ---
        wu_w = const_pool.tile([P, P], bf16, name="wu_w")
        nc.vector.memset(wu_w[:], 0.0)
        wu_ps = ctx.enter_context(
            tc.tile_pool(name="warm", bufs=1, space="PSUM")
        ).tile([P, P], f32, name="wu_ps")
        for i in range(30):
            nc.tensor.matmul(wu_ps[:], wu_w[:], wu_w[:], start=True, stop=True)

        # --- first deps of the PE, split across the two HWDGE dispatch
        # queues (SP + ACT), ordered by consumption time: wa[j0] + x first,
        # then wb[j0], then the a/b bulk in two interleaved halves ---
        wa0 = wpool.tile([P, D], bf16, tag="wa0", name="wa0")
        waR = wpool.tile([P, KT - 1, D], bf16, tag="waR", name="waR")
        wb0 = wpool.tile([P, D], bf16, tag="wb0", name="wb0")
        wbR = wpool.tile([P, KT - 1, D], bf16, tag="wbR", name="wbR")
        nc.sync.dma_start(wa0[:], wa_d[:, 0, :])
        xt0 = xt_pool.tile([P, KT, TB], bf16, tag="xt", name="xt0")
        nc.sync.dma_start(xt0[:], xt_d[:, 0])
        nc.scalar.dma_start(wb0[:], wb_d[:, 0, :])
        nc.scalar.dma_start(waR[:], wa_d[:, 1:KT, :])
        nc.scalar.dma_start(wbR[:], wb_d[:, 1:KT, :])

        def wa_slice(j, k):
            if j == 0:
                return wa0[:, k * P : (k + 1) * P]
            return waR[:, j - 1, k * P : (k + 1) * P]

        def wb_slice(j, k):
            if j == 0:
                return wb0[:, k * P : (k + 1) * P]
            return wbR[:, j - 1, k * P : (k + 1) * P]

        ba_sb = const_pool.tile([P, KT], f32, name="ba")
        nc.sync.dma_start(ba_sb[:], ba_d[:])
        # sigmoid(z) = 0.5 + 0.5*tanh(z/2): pre-halve the bias, keep every
        # ACT op (Tanh/Gelu_apprx_tanh/Copy) on one activation table.
        bah = const_pool.tile([P, KT], f32, name="bah")
        nc.scalar.mul(bah[:], ba_sb[:], 0.5)
        h0_sb = const_pool.tile([P, KT], f32, name="h0")
        nc.sync.dma_start(h0_sb[:], h0_d[:])

        # y-phase weights: dispatched up front on the ACT queue, after the
        # a/b bulk (consumption order: first needed ~35us in, at Y(0);
        # skip-path first so ws before wy)
        wy_sb = wpool.tile([P, KT, D], bf16, tag="wy", name="wy")
        ws_sb = wpool.tile([P, KT, D], bf16, tag="ws", name="ws")
        nc.scalar.dma_start(ws_sb[:], ws_d[:])
        nc.scalar.dma_start(wy_sb[:], wy_d[:])

        hT_prev = [None] * KT   # previous block's hT tiles (carry source)
        pend = None             # (xT, gT list) of previous block

        for n in range(NB + 1):
            if n < NB:
                t0 = n * TB
                if n == 0:
                    xT = xt0
                else:
                    xT = xt_pool.tile([P, KT, TB], bf16, tag="xt", name=f"xt{n}")
                    nc.sync.dma_start(xT[:], xt_d[:, n])

                # --- a/b matmuls + sigmoid + scan + gelu, per j ---
                hT_cur, gT_cur = [], []
                for j in range(KT):
                    psA = ps_ab.tile([P, TB], f32, tag="a", name=f"psA{n}_{j}")
                    psB = ps_ab.tile([P, TB], f32, tag="b", name=f"psB{n}_{j}")
                    for k in range(KT):
                        nc.tensor.matmul(
                            psA[:],
                            wa_slice(j, k),
                            xT[:, k, :],
                            start=(k == 0),
                            stop=(k == KT - 1),
                        )
                    for k in range(KT):
                        nc.tensor.matmul(
                            psB[:],
                            wb_slice(j, k),
                            xT[:, k, :],
                            start=(k == 0),
                            stop=(k == KT - 1),
                        )
                    # sigmoid(z+ba) = 0.5 + 0.5*tanh(0.5*z + 0.5*ba)
                    aT = sc_pool.tile([P, TB], f32, tag="aT", name=f"aT{n}_{j}")
                    nc.scalar.activation(
                        aT[:], psA[:], AF.Tanh, bias=bah[:, j : j + 1], scale=0.5
                    )
                    nc.gpsimd.tensor_scalar(
                        aT[:], aT[:], 0.5, 0.5, op0=ALU.mult, op1=ALU.add
                    )
                    hT = sc_pool.tile([P, TB], f32, tag=f"hT{j}", name=f"hT{n}_{j}")
                    init = (
                        h0_sb[:, j : j + 1]
                        if n == 0
                        else hT_prev[j][:, TB - 1 : TB]
                    )
                    nc.vector.tensor_tensor_scan(
                        hT[:], aT[:], psB[:], init, op0=ALU.mult, op1=ALU.add
                    )
                    gT = sc_pool.tile([P, TB], bf16, tag=f"gT{j}", name=f"gT{n}_{j}")
                    nc.scalar.activation(gT[:], hT[:], AF.Gelu_apprx_tanh)
                    hb = st_pool.tile([P, TB], bf16, tag="hb", name=f"hb{n}_{j}")
                    nc.scalar.copy(hb[:], hT[:])
                    nc.sync.dma_start(h_d[:, j, t0 : t0 + TB], hb[:])
                    hT_cur.append(hT)
                    gT_cur.append(gT)

            if n >= 1:
                # --- y matmuls for block n-1 (gives the scan tail slack) ---
                xT_p, gT_p = pend
                t0p = (n - 1) * TB
                for o in range(KT):
                    psY = ps_y.tile([P, TB], f32, tag="y", name=f"psY{n-1}_{o}")
                    # skip-path first: no dependency on the scan output
                    for k in range(KT):
                        nc.tensor.matmul(
                            psY[:],
                            ws_sb[:, k, o * P : (o + 1) * P],
                            xT_p[:, k, :],
                            start=(k == 0),
                            stop=False,
                        )
                    for j in range(KT):
                        nc.tensor.matmul(
                            psY[:],
                            wy_sb[:, j, o * P : (o + 1) * P],
                            gT_p[j][:],
                            start=False,
                            stop=(j == KT - 1),
                        )
                    yb = st_pool.tile([P, TB], bf16, tag="yb", name=f"yb{n-1}_{o}")
                    nc.vector.tensor_copy(yb[:], psY[:])
                    nc.sync.dma_start(y_d[:, o, t0p : t0p + TB], yb[:])

            if n < NB:
                pend = (xT, gT_cur)
                hT_prev = hT_cur

    nc.finalize()
    return nc


def _prep_weights(W_a, b_a, W_b, W_y, W_skip):
    """Host-side: tile + cast weights once (shared across cores)."""
    def tile_w(W):
        # [D, D] -> [p, k, e] with d_in = k*128 + p (contraction-major)
        return np.ascontiguousarray(
            np.asarray(W, dtype=np.float32)
            .reshape(KT, P, D)
            .transpose(1, 0, 2)
            .astype(ml_dtypes.bfloat16)
        )

    def tile_w_j(W):
        # [D, D] -> [p, j, k*128+q] = W[k*128+p, j*128+q] (output-block-major)
        return np.ascontiguousarray(
            np.asarray(W, dtype=np.float32)
            .reshape(KT, P, KT, P)
            .transpose(1, 2, 0, 3)
            .reshape(P, KT, D)
            .astype(ml_dtypes.bfloat16)
        )

    ba_t = np.ascontiguousarray(
        np.asarray(b_a, dtype=np.float32).reshape(KT, P).T
    )
    return {
        "wa": tile_w_j(W_a),
        "wb": tile_w_j(W_b),
        "wy": tile_w(W_y),
        "ws": tile_w(W_skip),
        "ba": ba_t,
    }


def kernel(x, h0, W_a, b_a, W_b, W_y, W_skip):
    if "nc" not in _CACHE:
        _CACHE["nc"] = _build()
    nc = _CACHE["nc"]

    shared = _prep_weights(W_a, b_a, W_b, W_y, W_skip)
    x = np.asarray(x, dtype=np.float32)
    h0 = np.asarray(h0, dtype=np.float32)

    in_maps = []
    for b in range(B):
        # x[b]: [T, D] -> xt[p, n, k, t] bf16 (block-contiguous)
        xt = np.ascontiguousarray(
            x[b].T.reshape(KT, P, NB, TB)
            .transpose(1, 2, 0, 3)
            .astype(ml_dtypes.bfloat16)
        )
        h0_t = np.ascontiguousarray(h0[b].reshape(KT, P).T)
        in_maps.append({"xt": xt, "h0": h0_t, **shared})

    res = run_bass_kernel_spmd(nc, in_maps, core_ids=list(range(B)))

    h = np.empty((B, T, D), np.float32)
    y = np.empty((B, T, D), np.float32)
    for b, r in enumerate(res.results):
        # [p, j, t] -> [t, j*128+p]
        h[b] = r["h_out"].astype(np.float32).transpose(1, 0, 2).reshape(D, T).T
        y[b] = r["y_out"].astype(np.float32).transpose(1, 0, 2).reshape(D, T).T
    return h, y


# revision 27
# speedup vs baseline: 1.0335x; 1.0335x over previous
"""Trainium2 Bass kernel for the Memoroid linear-recurrence block.

Math (per batch b, fp32):
    a = sigmoid(x @ W_a + b_a)          [T, D]
    bm = x @ W_b                        [T, D]
    h_t = a_t * h_{t-1} + bm_t          (h_{-1} = h0, scan over t)
    y = gelu_tanh(h) @ W_y + x @ W_skip [T, D]
Returns (h, y).

Strategy: data-parallel over batch (8 sequences -> 8 cores). All on-chip
work happens in the transposed [d, t] orientation so the recurrence maps
onto the DVE tensor_tensor_scan instruction and NO PE transposes are
needed anywhere:
  - the host ships x already transposed (and cast to bf16) as
    [128 part, 8 k, T] so it is directly usable as the matmul moving
    operand ([d_in, t] tiles),
  - a/b are computed as [d_h, t] tiles (weights stationary),
  - y is computed transposed as well: yT[d_out, t] = W_y^T gelu(h)T +
    W_skip^T xT, consuming the scan output gT in its native layout,
  - h/y are stored transposed+bf16 to DRAM and the host transposes the
    fp32 result back.
All matmuls run in bf16 (1 cyc/row, fast weight loads); PSUM accumulates
fp32 and the scan carry stays fp32, so the end-to-end max rel-err is
~3e-3 (dominated by bf16 input rounding).
"""

import sys

for _p in ("/opt/trn_rl_repo",):
    if _p not in sys.path:
        sys.path.insert(0, _p)

from contextlib import ExitStack

import numpy as np
import ml_dtypes

import concourse.bass as bass
import concourse.bacc as bacc
import concourse.mybir as mybir
from concourse import tile
from concourse.bass_utils import run_bass_kernel_spmd

B, T, D = 8, 4096, 1024
P = 128
KT = D // P            # 8 partition tiles along any d-dimension
TB = 512               # time-block length (matmul free dim / scan length)
NB = T // TB           # 8 blocks

f32 = mybir.dt.float32
bf16 = mybir.dt.bfloat16

_CACHE = {}


def _build():
    nc = bacc.Bacc()

    # x pre-transposed on host, block-contiguous so each block is one DMA
    # of 128 descriptors x 8KB: xt[p, n, k, t] = x[n*TB+t, k*128+p], bf16
    xt_d = nc.declare_dram_parameter("xt", [P, NB, KT, TB], bf16, False)
    # a/b weights pre-tiled on host OUTPUT-block-major so the j=0 block is
    # one small DMA and the PE can start after ~1.25 MB instead of 5 MB:
    #   w[p, j, k*128+q] = W[k*128+p, j*128+q]
    wa_d = nc.declare_dram_parameter("wa", [P, KT, D], bf16, False)
    wb_d = nc.declare_dram_parameter("wb", [P, KT, D], bf16, False)
    # y/skip weights contraction-major (loaded later, one DMA each):
    #   w[p, k, e] = W[k*128+p, e]
    wy_d = nc.declare_dram_parameter("wy", [P, KT, D], bf16, False)
    ws_d = nc.declare_dram_parameter("ws", [P, KT, D], bf16, False)
    # bias/initial state tiled [p, j] fp32
    ba_d = nc.declare_dram_parameter("ba", [P, KT], f32, False)
    h0_d = nc.declare_dram_parameter("h0", [P, KT], f32, False)
    # outputs transposed: [p, j, t] bf16; host transposes back
    h_d = nc.declare_dram_parameter("h_out", [P, KT, T], bf16, True)
    y_d = nc.declare_dram_parameter("y_out", [P, KT, T], bf16, True)

    AF = mybir.ActivationFunctionType
    ALU = mybir.AluOpType

    with tile.TileContext(nc) as tc, ExitStack() as ctx:
        wpool = ctx.enter_context(tc.tile_pool(name="weights", bufs=1))
        const_pool = ctx.enter_context(tc.tile_pool(name="const", bufs=1))
        xt_pool = ctx.enter_context(tc.tile_pool(name="xt", bufs=3))
        sc_pool = ctx.enter_context(tc.tile_pool(name="scan", bufs=2))
        st_pool = ctx.enter_context(tc.tile_pool(name="stage", bufs=2))
        ps_ab = ctx.enter_context(tc.tile_pool(name="ab", bufs=2, space="PSUM"))
        ps_y = ctx.enter_context(tc.tile_pool(name="ypsum", bufs=3, space="PSUM"))

        # --- PE warm-up: short matmuls on zeroed scratch, no DMA deps, so
        # the HAM clock gate is at 2.4 GHz by the time real matmuls start.
        # FD=128 keeps each one ~107ns so the queue drains before real
        # work is ready. ---
        wu_w = const_pool.tile([P, P], bf16, name="wu_w")
        nc.vector.memset(wu_w[:], 0.0)
        wu_ps = ctx.enter_context(
            tc.tile_pool(name="warm", bufs=1, space="PSUM")
        ).tile([P, P], f32, name="wu_ps")
        for i in range(30):
            nc.tensor.matmul(wu_ps[:], wu_w[:], wu_w[:], start=True, stop=True)

        # --- first deps of the PE, split across the two HWDGE dispatch
        # queues (SP + ACT), ordered by consumption time: wa[j0] + x first,
        # then wb[j0], then the a/b bulk in two interleaved halves ---
        wa0 = wpool.tile([P, D], bf16, tag="wa0", name="wa0")
        waR = wpool.tile([P, KT - 1, D], bf16, tag="waR", name="waR")
        wb0 = wpool.tile([P, D], bf16, tag="wb0", name="wb0")
        wbR = wpool.tile([P, KT - 1, D], bf16, tag="wbR", name="wbR")
        # SP carries the startup-critical stream in consumption order; ACT
        # only dispatches wb0 so its queue is free for activation compute.
        nc.sync.dma_start(wa0[:], wa_d[:, 0, :])
        xt0 = xt_pool.tile([P, KT, TB], bf16, tag="xt", name="xt0")
        nc.sync.dma_start(xt0[:], xt_d[:, 0])
        nc.scalar.dma_start(wb0[:], wb_d[:, 0, :])
        H = (KT - 1) // 2
        nc.sync.dma_start(waR[:, 0:H, :], wa_d[:, 1 : 1 + H, :])
        nc.sync.dma_start(wbR[:, 0:H, :], wb_d[:, 1 : 1 + H, :])
        nc.sync.dma_start(waR[:, H : KT - 1, :], wa_d[:, 1 + H : KT, :])
        nc.sync.dma_start(wbR[:, H : KT - 1, :], wb_d[:, 1 + H : KT, :])

        def wa_slice(j, k):
            if j == 0:
                return wa0[:, k * P : (k + 1) * P]
            return waR[:, j - 1, k * P : (k + 1) * P]

        def wb_slice(j, k):
            if j == 0:
                return wb0[:, k * P : (k + 1) * P]
            return wbR[:, j - 1, k * P : (k + 1) * P]

        ba_sb = const_pool.tile([P, KT], f32, name="ba")
        nc.sync.dma_start(ba_sb[:], ba_d[:])
        # sigmoid(z) = 0.5 + 0.5*tanh(z/2): pre-halve the bias, keep every
        # ACT op (Tanh/Gelu_apprx_tanh/Copy) on one activation table.
        bah = const_pool.tile([P, KT], f32, name="bah")
        nc.scalar.mul(bah[:], ba_sb[:], 0.5)
        h0_sb = const_pool.tile([P, KT], f32, name="h0")
        nc.sync.dma_start(h0_sb[:], h0_d[:])

        # y-phase weights: dispatched on SP behind the a/b bulk (consumption
        # order: first needed ~38us in, at Y(0); skip-path MMs come first
        # so ws before wy)
        wy_sb = wpool.tile([P, KT, D], bf16, tag="wy", name="wy")
        ws_sb = wpool.tile([P, KT, D], bf16, tag="ws", name="ws")
        nc.sync.dma_start(ws_sb[:], ws_d[:])
        nc.sync.dma_start(wy_sb[:], wy_d[:])

        hT_prev = [None] * KT   # previous block's hT tiles (carry source)
        pend = None             # (xT, gT list) of previous block
        xt_next = xt0           # block-n x tile, prefetched one block ahead

        for n in range(NB + 1):
            if n < NB:
                t0 = n * TB
                xT = xt_next
                if n + 1 < NB:
                    xt_next = xt_pool.tile(
                        [P, KT, TB], bf16, tag="xt", name=f"xt{n+1}"
                    )
                    nc.sync.dma_start(xt_next[:], xt_d[:, n + 1])

                # --- a/b matmuls + sigmoid + scan + gelu, per j ---
                hT_cur, gT_cur = [], []
                for j in range(KT):
                    psA = ps_ab.tile([P, TB], f32, tag="a", name=f"psA{n}_{j}")
                    psB = ps_ab.tile([P, TB], f32, tag="b", name=f"psB{n}_{j}")
                    for k in range(KT):
                        nc.tensor.matmul(
                            psA[:],
                            wa_slice(j, k),
                            xT[:, k, :],
                            start=(k == 0),
                            stop=(k == KT - 1),
                        )
                    for k in range(KT):
                        nc.tensor.matmul(
                            psB[:],
                            wb_slice(j, k),
                            xT[:, k, :],
                            start=(k == 0),
                            stop=(k == KT - 1),
                        )
                    # sigmoid(z+ba) = 0.5 + 0.5*tanh(0.5*z + 0.5*ba)
                    aT = sc_pool.tile([P, TB], f32, tag="aT", name=f"aT{n}_{j}")
                    nc.scalar.activation(
                        aT[:], psA[:], AF.Tanh, bias=bah[:, j : j + 1], scale=0.5
                    )
                    nc.gpsimd.tensor_scalar(
                        aT[:], aT[:], 0.5, 0.5, op0=ALU.mult, op1=ALU.add
                    )
                    hT = sc_pool.tile([P, TB], f32, tag=f"hT{j}", name=f"hT{n}_{j}")
                    init = (
                        h0_sb[:, j : j + 1]
                        if n == 0
                        else hT_prev[j][:, TB - 1 : TB]
                    )
                    nc.vector.tensor_tensor_scan(
                        hT[:], aT[:], psB[:], init, op0=ALU.mult, op1=ALU.add
                    )
                    gT = sc_pool.tile([P, TB], bf16, tag=f"gT{j}", name=f"gT{n}_{j}")
                    nc.scalar.activation(gT[:], hT[:], AF.Gelu_apprx_tanh)
                    hb = st_pool.tile([P, TB], bf16, tag="hb", name=f"hb{n}_{j}")
                    nc.scalar.copy(hb[:], hT[:])
                    nc.sync.dma_start(h_d[:, j, t0 : t0 + TB], hb[:])
                    hT_cur.append(hT)
                    gT_cur.append(gT)

            if n >= 1:
                # --- y matmuls for block n-1 (gives the scan tail slack) ---
                xT_p, gT_p = pend
                t0p = (n - 1) * TB
                for o in range(KT):
                    psY = ps_y.tile([P, TB], f32, tag="y", name=f"psY{n-1}_{o}")
                    # skip-path first: no dependency on the scan output
                    for k in range(KT):
                        nc.tensor.matmul(
                            psY[:],
                            ws_sb[:, k, o * P : (o + 1) * P],
                            xT_p[:, k, :],
                            start=(k == 0),
                            stop=False,
                        )
                    for j in range(KT):
                        nc.tensor.matmul(
                            psY[:],
                            wy_sb[:, j, o * P : (o + 1) * P],
                            gT_p[j][:],
                            start=False,
                            stop=(j == KT - 1),
                        )
                    yb = st_pool.tile([P, TB], bf16, tag="yb", name=f"yb{n-1}_{o}")
                    nc.vector.tensor_copy(yb[:], psY[:])
                    nc.sync.dma_start(y_d[:, o, t0p : t0p + TB], yb[:])

            if n < NB:
                pend = (xT, gT_cur)
                hT_prev = hT_cur

    nc.finalize()
    return nc


def _prep_weights(W_a, b_a, W_b, W_y, W_skip):
    """Host-side: tile + cast weights once (shared across cores)."""
    def tile_w(W):
        # [D, D] -> [p, k, e] with d_in = k*128 + p (contraction-major)
        return np.ascontiguousarray(
            np.asarray(W, dtype=np.float32)
            .reshape(KT, P, D)
            .transpose(1, 0, 2)
            .astype(ml_dtypes.bfloat16)
        )

    def tile_w_j(W):
        # [D, D] -> [p, j, k*128+q] = W[k*128+p, j*128+q] (output-block-major)
        return np.ascontiguousarray(
            np.asarray(W, dtype=np.float32)
            .reshape(KT, P, KT, P)
            .transpose(1, 2, 0, 3)
            .reshape(P, KT, D)
            .astype(ml_dtypes.bfloat16)
        )

    ba_t = np.ascontiguousarray(
        np.asarray(b_a, dtype=np.float32).reshape(KT, P).T
    )
    return {
        "wa": tile_w_j(W_a),
        "wb": tile_w_j(W_b),
        "wy": tile_w(W_y),
        "ws": tile_w(W_skip),
        "ba": ba_t,
    }


def kernel(x, h0, W_a, b_a, W_b, W_y, W_skip):
    if "nc" not in _CACHE:
        _CACHE["nc"] = _build()
    nc = _CACHE["nc"]

    shared = _prep_weights(W_a, b_a, W_b, W_y, W_skip)
    x = np.asarray(x, dtype=np.float32)
    h0 = np.asarray(h0, dtype=np.float32)

    in_maps = []
    for b in range(B):
        # x[b]: [T, D] -> xt[p, n, k, t] bf16 (block-contiguous)
        xt = np.ascontiguousarray(
            x[b].T.reshape(KT, P, NB, TB)
            .transpose(1, 2, 0, 3)
            .astype(ml_dtypes.bfloat16)
        )
        h0_t = np.ascontiguousarray(h0[b].reshape(KT, P).T)
        in_maps.append({"xt": xt, "h0": h0_t, **shared})

    res = run_bass_kernel_spmd(nc, in_maps, core_ids=list(range(B)))

    h = np.empty((B, T, D), np.float32)
    y = np.empty((B, T, D), np.float32)
    for b, r in enumerate(res.results):
        # [p, j, t] -> [t, j*128+p]
        h[b] = r["h_out"].astype(np.float32).transpose(1, 0, 2).reshape(D, T).T
        y[b] = r["y_out"].astype(np.float32).transpose(1, 0, 2).reshape(D, T).T
    return h, y
